# revision 8
# baseline (speedup 1.0000x reference)
"""Trainium2 Bass kernel for nn_Attention_49478023250074.

Multi-head causal attention (shortformer positional embedding variant):
  B=4, S=2048, D=768, H=12, Dh=64.

Sharding: 8 NeuronCores = data-parallel over batch (4) x tensor-parallel over
heads (2 groups of 6). Each core computes the full attention output
contribution of its 6 heads for its batch; the output projection partial sums
are reduced on the host (the "all-reduce" of the sharding hint).

On-core dataflow (all matmul operands fp16, accumulation fp32):
  xqkT = residT + posT                               (DVE, [768,2048] d-major)
  QT[pair] = (Wq2.T @ xqk).T-layout [e2=128, S]      (PE, per 2-head pair)
  KT[h]   = head-padded [128, S] ( K/8 rows in the head's 64-partition slot )
  V1      = [ones | V_h] blocks [S, 6*65]            (ones column -> denom row)
  per (head, 256-wide q-chunk):
     S^T[k,q] chunks = KT[h].T @ QT[pair]            (PE, causal blocks only)
     P = exp(S^T) (* causal mask on diagonal blocks) (ACT + DVE)
     zp = V1.T @ P -> [65, q]: row0 = sum_k P (denominator), rows 1:65 = z^T
     z^T *= 1/denom (broadcast via PE ones outer-product)
  out = z^T.T @ Wo + b_O/2                           (PE + DVE)
"""

import os
import sys

for _p in ("/opt/trn_rl_repo", "/root/.axon_site/_ro/trn_rl_repo"):
    if os.path.isdir(_p) and _p not in sys.path:
        sys.path.insert(0, _p)

from contextlib import ExitStack

import numpy as np

import concourse.bass as bass
import concourse.mybir as mybir
import concourse.tile as tile

B, S, D, H, Dh = 4, 2048, 768, 12, 64
HG = 6          # heads per core
NPAIR = 3       # head pairs per core
QC = 256        # attention q-chunk width
NJ = S // QC    # q-chunks
NKC = S // 128  # k-chunks
F16 = mybir.dt.float16
F32 = mybir.dt.float32
ADD = mybir.AluOpType.add
MULT = mybir.AluOpType.mult
EXP = mybir.ActivationFunctionType.Exp
N_CORES = 8


def _split_drain_waits(nc):
    """The walrus build in this environment accepts only ONE sync-wait per
    instruction (setupSyncWait raises 'Too many sync wait commands' for two or
    more). Peel extra waits onto no-ops placed just before the instruction on
    the same engine — same-engine program order keeps the semantics: the
    engine stalls on the peeled waits before reaching the instruction."""
    for f in nc.m.functions:
        for bb in f.blocks:
            insts = list(bb.instructions)
            out, changed = [], False
            for ins in insts:
                si = ins.sync_info
                if si is not None and len(si.on_wait) > 1:
                    waits = list(si.on_wait)
                    for i, w in enumerate(waits[:-1]):
                        out.append(
                            mybir.InstNoOp(
                                name=f"{ins.name}-w{i}",
                                engine=ins.engine,
                                sync_info=mybir.SyncInfo(on_wait=[w], on_update=[]),
                                bass_nofuse=True,
                            )
                        )
                    ins.sync_info = mybir.SyncInfo(
                        on_wait=[waits[-1]], on_update=list(si.on_update)
                    )
                    changed = True
                out.append(ins)
            if changed:
                bb.instructions.clear()
                for i2 in out:
                    bb.add_instruction(i2)


def build_nc(split_drains=True):
    nc = bass.Bass()
    residT = nc.dram_tensor("residT", [D, S], F16, kind="ExternalInput")
    posT = nc.dram_tensor("posT", [D, S], F16, kind="ExternalInput")
    wq = nc.dram_tensor("wq", [D, 384], F16, kind="ExternalInput")
    wk = nc.dram_tensor("wk", [D, 384], F16, kind="ExternalInput")  # pre /8
    wv = nc.dram_tensor("wv", [D, 384], F16, kind="ExternalInput")
    wo = nc.dram_tensor("wo", [384, D], F16, kind="ExternalInput")
    bq = nc.dram_tensor("bq", [128, NPAIR], F32, kind="ExternalInput")
    bk = nc.dram_tensor("bk", [128, NPAIR], F32, kind="ExternalInput")  # pre /8
    bv = nc.dram_tensor("bv", [128, 384], F32, kind="ExternalInput")
    bo = nc.dram_tensor("bo", [128, D], F32, kind="ExternalInput")  # pre /2
    maskA = nc.dram_tensor("maskA", [128, QC], F16, kind="ExternalInput")
    maskB = nc.dram_tensor("maskB", [128, QC], F16, kind="ExternalInput")
    out = nc.dram_tensor("out", [S, D], F32, kind="ExternalOutput")

    with tile.TileContext(nc) as tc, ExitStack() as ctx:
        const = ctx.enter_context(tc.tile_pool(name="const", bufs=1))
        psum_proj = ctx.enter_context(
            tc.tile_pool(name="psum_proj", bufs=3, space="PSUM")
        )
        psum_st = ctx.enter_context(tc.tile_pool(name="psum_st", bufs=2, space="PSUM"))
        psum_z = ctx.enter_context(tc.tile_pool(name="psum_z", bufs=2, space="PSUM"))
        psum_bc = ctx.enter_context(tc.tile_pool(name="psum_bc", bufs=1, space="PSUM"))
        pt_pool = ctx.enter_context(tc.tile_pool(name="pt", bufs=6))
        rc_pool = ctx.enter_context(tc.tile_pool(name="rc", bufs=2))
        zs_pool = ctx.enter_context(tc.tile_pool(name="zs", bufs=3))
        out_pool = ctx.enter_context(tc.tile_pool(name="outp", bufs=3))

        residT_sb = const.tile([128, 6, S], F16)
        posT_sb = const.tile([128, 6, S], F16)
        xqkT_sb = const.tile([128, 6, S], F16)
        wq_sb = const.tile([128, 6, 384], F16)
        wk_sb = const.tile([128, 6, 384], F16)
        wv_sb = const.tile([128, 6, 384], F16)
        wo_sb = const.tile([128, 3, D], F16)
        bq_sb = const.tile([128, NPAIR], F32)
        bk_sb = const.tile([128, NPAIR], F32)
        bv_sb = const.tile([128, 384], F32)
        bo_sb = const.tile([128, D], F32)
        mA_sb = const.tile([128, QC], F16)
        mB_sb = const.tile([128, QC], F16)
        qt_sb = const.tile([128, NPAIR, S], F16)
        kt_sb = const.tile([128, HG, S], F16)
        v1_sb = const.tile([128, NKC, HG * 65], F16)
        zt2_sb = const.tile([128, NPAIR, S], F16)
        # ones row at partition 64 so the K=1 broadcast matmul's operands sit
        # in the same partition window as the denominator row (offset 64)
        ones_sb = const.tile([65, 128], F32)

        # ---- loads + init ----
        for dc in range(6):
            nc.sync.dma_start(residT_sb[:, dc, :], residT[128 * dc : 128 * dc + 128, :])
            nc.sync.dma_start(posT_sb[:, dc, :], posT[128 * dc : 128 * dc + 128, :])
            nc.sync.dma_start(wq_sb[:, dc, :], wq[128 * dc : 128 * dc + 128, :])
            nc.sync.dma_start(wk_sb[:, dc, :], wk[128 * dc : 128 * dc + 128, :])
            nc.sync.dma_start(wv_sb[:, dc, :], wv[128 * dc : 128 * dc + 128, :])
        for t in range(3):
            nc.sync.dma_start(wo_sb[:, t, :], wo[128 * t : 128 * t + 128, :])
        nc.sync.dma_start(bq_sb[:], bq[:])
        nc.sync.dma_start(bk_sb[:], bk[:])
        nc.sync.dma_start(bv_sb[:], bv[:])
        nc.sync.dma_start(bo_sb[:], bo[:])
        nc.sync.dma_start(mA_sb[:], maskA[:])
        nc.sync.dma_start(mB_sb[:], maskB[:])
        nc.vector.memset(kt_sb[:], 0.0)
        nc.vector.memset(v1_sb[:], 1.0)
        nc.vector.memset(ones_sb[:], 1.0)

        # ---- phase A: xqk = resid + pos (fp16) ----
        for dc in range(6):
            nc.vector.tensor_add(
                xqkT_sb[:, dc, :], residT_sb[:, dc, :], posT_sb[:, dc, :]
            )

        # ---- phase B: projections ----
        for p in range(NPAIR):
            h0, h1 = 2 * p, 2 * p + 1
            for n in range(4):
                sl = slice(512 * n, 512 * n + 512)
                psq = psum_proj.tile([128, 512], F32, tag="proj")
                for dc in range(6):
                    nc.tensor.matmul(
                        psq,
                        lhsT=wq_sb[:, dc, 128 * p : 128 * p + 128],
                        rhs=xqkT_sb[:, dc, sl],
                        start=dc == 0,
                        stop=dc == 5,
                    )
                nc.vector.tensor_scalar(
                    qt_sb[:, p, sl], psq, bq_sb[:, p : p + 1], None, ADD
                )
                psk = psum_proj.tile([128, 512], F32, tag="proj")
                for dc in range(6):
                    nc.tensor.matmul(
                        psk,
                        lhsT=wk_sb[:, dc, 128 * p : 128 * p + 128],
                        rhs=xqkT_sb[:, dc, sl],
                        start=dc == 0,
                        stop=dc == 5,
                    )
                nc.vector.tensor_scalar(
                    kt_sb[0:64, h0, sl], psk[0:64, :], bk_sb[0:64, p : p + 1], None, ADD
                )
                nc.vector.tensor_scalar(
                    kt_sb[64:128, h1, sl],
                    psk[64:128, :],
                    bk_sb[64:128, p : p + 1],
                    None,
                    ADD,
                )

        for n in range(NKC):
            psv = psum_proj.tile([128, 512], F32, tag="proj")
            for dc in range(6):
                nc.tensor.matmul(
                    psv[:, :384],
                    lhsT=residT_sb[:, dc, 128 * n : 128 * n + 128],
                    rhs=wv_sb[:, dc, :],
                    start=dc == 0,
                    stop=dc == 5,
                )
            for h in range(HG):
                nc.vector.tensor_tensor(
                    v1_sb[:, n, 65 * h : 65 * h + 64],
                    psv[:, 64 * h : 64 * h + 64],
                    bv_sb[:, 64 * h : 64 * h + 64],
                    ADD,
                )

        # ---- phase C: attention ----
        for h in range(HG):
            p, s = h // 2, h % 2
            # even heads write their zt2 slot (partitions 0:64) directly;
            # odd heads stage at partitions 0:64 and DMA-shift to 64:128
            if s == 1:
                zstag = zs_pool.tile([64, S], F16, tag="zstag", name="zstag")
            else:
                zstag = None
            for j in range(NJ):
                qsl = slice(QC * j, QC * j + QC)
                zp = psum_z.tile([65, QC], F32, tag="z")
                last = 2 * j + 1
                for kc in range(2 * j + 2):
                    sp = psum_st.tile([128, QC], F32, tag="st")
                    nc.tensor.matmul(
                        sp,
                        lhsT=kt_sb[:, h, 128 * kc : 128 * kc + 128],
                        rhs=qt_sb[:, p, qsl],
                        start=True,
                        stop=True,
                    )
                    pt = pt_pool.tile([128, QC], F16, tag="pt")
                    nc.scalar.activation(pt, sp, EXP)
                    if kc == 2 * j:
                        nc.vector.tensor_mul(pt, pt, mA_sb)
                    elif kc == last:
                        nc.vector.tensor_mul(pt, pt, mB_sb)
                    nc.tensor.matmul(
                        zp,
                        lhsT=v1_sb[:, kc, 65 * h : 65 * h + 65],
                        rhs=pt,
                        start=kc == 0,
                        stop=kc == last,
                    )
                rc_t = rc_pool.tile([65, QC], F32, tag="rc")
                rc = rc_t[64:65, :]
                nc.vector.reciprocal(rc, zp[64:65, :])
                bc = psum_bc.tile([128, QC], F32, tag="bc")
                nc.tensor.matmul(
                    bc, lhsT=ones_sb[64:65, :], rhs=rc, start=True, stop=True
                )
                dst = zstag[:, qsl] if s == 1 else zt2_sb[0:64, p, qsl]
                nc.vector.tensor_copy(dst, zp[0:64, :])
                nc.vector.tensor_mul(dst, dst, bc[0:64, :])
            if s == 1:
                # partition-shift into the pair-stacked layout for the out-proj
                nc.sync.dma_start(zt2_sb[64:128, p, :], zstag[:, :])

        # ---- phase D: output projection ----
        for n in range(NKC):
            ps0 = psum_proj.tile([128, 512], F32, tag="proj")
            ps1 = psum_proj.tile([128, 512], F32, tag="proj")
            for t in range(3):
                nc.tensor.matmul(
                    ps0[:, :384],
                    lhsT=zt2_sb[:, t, 128 * n : 128 * n + 128],
                    rhs=wo_sb[:, t, 0:384],
                    start=t == 0,
                    stop=t == 2,
                )
            for t in range(3):
                nc.tensor.matmul(
                    ps1[:, :384],
                    lhsT=zt2_sb[:, t, 128 * n : 128 * n + 128],
                    rhs=wo_sb[:, t, 384:768],
                    start=t == 0,
                    stop=t == 2,
                )
            ob = out_pool.tile([128, D], F32, tag="out")
            nc.vector.tensor_add(ob[:, 0:384], ps0[:, :384], bo_sb[:, 0:384])
            nc.vector.tensor_add(ob[:, 384:768], ps1[:, :384], bo_sb[:, 384:768])
            nc.sync.dma_start(out[128 * n : 128 * n + 128, :], ob)

    if split_drains:
        _split_drain_waits(nc)
    return nc


def pack_inputs(
    resid_pre, shortformer_pos_embed, W_Q, W_K, W_V, W_O, b_Q, b_K, b_V, b_O
):
    """Build the 8 per-core input maps (host-side shard + transpose + cast)."""
    resid_pre = np.asarray(resid_pre, dtype=np.float32)
    pos = np.asarray(shortformer_pos_embed, dtype=np.float32)
    W_Q = np.asarray(W_Q, dtype=np.float32)
    W_K = np.asarray(W_K, dtype=np.float32)
    W_V = np.asarray(W_V, dtype=np.float32)
    W_O = np.asarray(W_O, dtype=np.float32)
    b_Q = np.asarray(b_Q, dtype=np.float32)
    b_K = np.asarray(b_K, dtype=np.float32)
    b_V = np.asarray(b_V, dtype=np.float32)
    b_O = np.asarray(b_O, dtype=np.float32)

    i_a = np.arange(128)[:, None]
    i_q = np.arange(QC)[None, :]
    maskA = (i_q >= i_a).astype(np.float16)
    maskB = (i_q >= i_a + 128).astype(np.float16)

    in_maps = []
    for c in range(N_CORES):
        b, g = divmod(c, 2)
        hs = slice(HG * g, HG * g + HG)
        wq = np.concatenate([W_Q[h] for h in range(HG * g, HG * g + HG)], axis=1)
        wk = np.concatenate([W_K[h] / 8.0 for h in range(HG * g, HG * g + HG)], axis=1)
        wv = np.concatenate([W_V[h] for h in range(HG * g, HG * g + HG)], axis=1)
        wo = W_O[hs].reshape(HG * Dh, D)
        bqc = b_Q[hs].reshape(NPAIR, 128).T  # [128, 3]; col p = pair p biases
        bkc = (b_K[hs] / 8.0).reshape(NPAIR, 128).T
        bvc = np.broadcast_to(b_V[hs].reshape(1, HG * Dh), (128, HG * Dh))
        boc = np.broadcast_to(b_O[None, :] * 0.5, (128, D))
        in_maps.append(
            {
                "residT": np.ascontiguousarray(resid_pre[b].T).astype(np.float16),
                "posT": np.ascontiguousarray(pos[b].T).astype(np.float16),
                "wq": wq.astype(np.float16),
                "wk": wk.astype(np.float16),
                "wv": wv.astype(np.float16),
                "wo": wo.astype(np.float16),
                "bq": np.ascontiguousarray(bqc),
                "bk": np.ascontiguousarray(bkc),
                "bv": np.ascontiguousarray(bvc),
                "bo": np.ascontiguousarray(boc),
                "maskA": maskA,
                "maskB": maskB,
            }
        )
    return in_maps


class Runner:
    """Compiles the Bass module once (via the bass2jax PJRT path that
    bass_utils.run_bass_kernel_spmd uses under axon) and keeps the jitted
    sharded executable for reuse."""

    def __init__(self):
        import jax
        from jax.sharding import Mesh, PartitionSpec
        from jax.experimental.shard_map import shard_map
        from concourse import bass2jax

        self.jax = jax
        bass2jax.install_neuronx_cc_hook()
        nc = build_nc()
        self.nc = nc

        partition_name = (
            nc.partition_id_tensor.name if nc.partition_id_tensor else None
        )
        in_names, out_names, out_avals, zero_outs = [], [], [], []
        for alloc in nc.m.functions[0].allocations:
            if not isinstance(alloc, mybir.MemoryLocationSet):
                continue
            name = alloc.memorylocations[0].name
            if alloc.kind == "ExternalInput":
                if name != partition_name:
                    in_names.append(name)
            elif alloc.kind == "ExternalOutput":
                shape = tuple(alloc.tensor_shape)
                dtype = mybir.dt.np(alloc.dtype)
                out_names.append(name)
                out_avals.append(jax.core.ShapedArray(shape, dtype))
                zero_outs.append(np.zeros(shape, dtype))
        self.in_names = list(in_names)
        self.out_names = out_names
        self.out_avals = out_avals
        self.zero_outs = zero_outs
        n_params = len(in_names)
        n_outs = len(out_names)
        all_in_names = in_names + out_names
        if partition_name is not None:
            all_in_names.append(partition_name)

        def _body(*args):
            operands = list(args)
            if partition_name is not None:
                operands.append(bass2jax.partition_id_tensor())
            outs = bass2jax._bass_exec_p.bind(
                *operands,
                out_avals=tuple(out_avals),
                in_names=tuple(all_in_names),
                out_names=tuple(out_names),
                lowering_input_output_aliases=(),
                sim_require_finite=True,
                sim_require_nnan=True,
                nc=nc,
            )
            return tuple(outs)

        devices = jax.devices()[:N_CORES]
        assert len(devices) == N_CORES, f"need {N_CORES} cores, have {len(devices)}"
        mesh = Mesh(np.asarray(devices), ("core",))
        in_specs = (PartitionSpec("core"),) * (n_params + n_outs)
        out_specs = (PartitionSpec("core"),) * n_outs
        donate = tuple(range(n_params, n_params + n_outs))
        self.sharded = jax.jit(
            shard_map(
                _body,
                mesh=mesh,
                in_specs=in_specs,
                out_specs=out_specs,
                check_rep=False,
            ),
            donate_argnums=donate,
            keep_unused=True,
        )

    def concat_inputs(self, in_maps):
        return [
            np.concatenate([np.asarray(m[name]) for m in in_maps], axis=0)
            for name in self.in_names
        ]

    def concat_zeros(self):
        return [
            np.zeros((N_CORES * z.shape[0], *z.shape[1:]), z.dtype)
            for z in self.zero_outs
        ]

    def run_concat(self, concat_in):
        """One sharded execution; returns the concatenated 'out' array
        [8*2048, 768] (per-core blocks along axis 0)."""
        out_arrs = self.sharded(*concat_in, *self.concat_zeros())
        return np.asarray(out_arrs[0])

    def run(self, in_maps):
        flat = self.run_concat(self.concat_inputs(in_maps))
        return flat.reshape(N_CORES, S, D)


_RUNNER = None


def get_runner():
    global _RUNNER
    if _RUNNER is None:
        _RUNNER = Runner()
    return _RUNNER


def kernel(**inputs):
    runner = get_runner()
    in_maps = pack_inputs(**inputs)
    per_core = runner.run(in_maps)
    # host all-reduce of the two head-group partials per batch
    out = per_core[0::2] + per_core[1::2]  # [4, 2048, 768]
    return out.astype(np.float32)


if __name__ == "__main__":
    # quick self-run with random inputs
    rng = np.random.default_rng(0)
    ins = {
        "resid_pre": rng.standard_normal((B, S, D), dtype=np.float32),
        "shortformer_pos_embed": rng.standard_normal((B, S, D), dtype=np.float32),
        "W_Q": rng.standard_normal((H, D, Dh), dtype=np.float32) * 0.02,
        "W_K": rng.standard_normal((H, D, Dh), dtype=np.float32) * 0.02,
        "W_V": rng.standard_normal((H, D, Dh), dtype=np.float32) * 0.02,
        "W_O": rng.standard_normal((H, Dh, D), dtype=np.float32) * 0.02,
        "b_Q": np.zeros((H, Dh), np.float32),
        "b_K": np.zeros((H, Dh), np.float32),
        "b_V": np.zeros((H, Dh), np.float32),
        "b_O": np.zeros((D,), np.float32),
    }
    out = kernel(**ins)
    print("out", out.shape, out.dtype, float(np.abs(out).max()))


# revision 32
# speedup vs baseline: 171.6693x; 171.6693x over previous
"""Trainium2 Bass kernel for nn_Attention_49478023250074.

Multi-head causal attention (shortformer positional embedding variant):
  B=4, S=2048, D=768, H=12, Dh=64.

Sharding: 8 NeuronCores = data-parallel over batch (4) x tensor-parallel over
heads (2 groups of 6). Each core computes the full attention output
contribution of its 6 heads for its batch; the output projection partial sums
are reduced on the host (the "all-reduce" of the sharding hint).

On-core dataflow (all matmul operands fp16, accumulation fp32):
  xqkT = residT + posT                               (DVE, [768,2048] d-major)
  QT[pair] = (Wq2.T @ xqk).T-layout [e2=128, S]      (PE, per 2-head pair)
  KT[h]   = head-padded [128, S] ( K/8 rows in the head's 64-partition slot )
  V1      = [ones | V_h] blocks [S, 6*65]            (ones column -> denom row)
  per (head, 256-wide q-chunk):
     S^T[k,q] chunks = KT[h].T @ QT[pair]            (PE, causal blocks only)
     P = exp(S^T) (* causal mask on diagonal blocks) (ACT + DVE)
     zp = V1.T @ P -> [65, q]: row0 = sum_k P (denominator), rows 1:65 = z^T
     z^T *= 1/denom (broadcast via PE ones outer-product)
  out = z^T.T @ Wo + b_O/2                           (PE + DVE)
"""

import os
import sys

for _p in ("/opt/trn_rl_repo", "/root/.axon_site/_ro/trn_rl_repo"):
    if os.path.isdir(_p) and _p not in sys.path:
        sys.path.insert(0, _p)

from contextlib import ExitStack

import numpy as np

import concourse.bass as bass
import concourse.mybir as mybir
import concourse.tile as tile
from concourse import library_config

B, S, D, H, Dh = 4, 2048, 768, 12, 64
HG = 6          # heads per core
NPAIR = 3       # head pairs per core
QC = 256        # attention q-chunk width
NJ = S // QC    # q-chunks
NKC = S // 128  # k-chunks
F16 = mybir.dt.float16
F32 = mybir.dt.float32
ADD = mybir.AluOpType.add
MULT = mybir.AluOpType.mult
EXP = mybir.ActivationFunctionType.Exp
N_CORES = 8


def _split_drain_waits(nc):
    """The walrus build in this environment accepts only ONE sync-wait per
    instruction (setupSyncWait raises 'Too many sync wait commands' for two or
    more). Peel extra waits onto no-ops placed just before the instruction on
    the same engine — same-engine program order keeps the semantics: the
    engine stalls on the peeled waits before reaching the instruction."""
    for f in nc.m.functions:
        for bb in f.blocks:
            insts = list(bb.instructions)
            out, changed = [], False
            for ins in insts:
                si = ins.sync_info
                if si is not None and len(si.on_wait) > 1:
                    waits = list(si.on_wait)
                    for i, w in enumerate(waits[:-1]):
                        out.append(
                            mybir.InstNoOp(
                                name=f"{ins.name}-w{i}",
                                engine=ins.engine,
                                sync_info=mybir.SyncInfo(on_wait=[w], on_update=[]),
                                bass_nofuse=True,
                            )
                        )
                    ins.sync_info = mybir.SyncInfo(
                        on_wait=[waits[-1]], on_update=list(si.on_update)
                    )
                    changed = True
                out.append(ins)
            if changed:
                bb.instructions.clear()
                for i2 in out:
                    bb.add_instruction(i2)


def build_nc(split_drains=True, reps=1, loop_reps=None):
    """loop_reps: wrap the compute phases in a tc.For_i hardware loop (for
    on-device timing: wall(loop_reps=N) - wall(loop_reps=None) ~= (N-1)*T)."""
    nc = bass.Bass()
    residT = nc.dram_tensor("residT", [D, S], F16, kind="ExternalInput")
    posT = nc.dram_tensor("posT", [D, S], F16, kind="ExternalInput")
    wq = nc.dram_tensor("wq", [D, 384], F16, kind="ExternalInput")
    wk = nc.dram_tensor("wk", [D, 384], F16, kind="ExternalInput")  # pre /8
    wv = nc.dram_tensor("wv", [D, 384], F16, kind="ExternalInput")
    wo = nc.dram_tensor("wo", [384, D], F16, kind="ExternalInput")
    bq = nc.dram_tensor("bq", [128, NPAIR], F32, kind="ExternalInput")
    bk = nc.dram_tensor("bk", [128, NPAIR], F32, kind="ExternalInput")  # pre /8
    bv = nc.dram_tensor("bv", [128, 384], F32, kind="ExternalInput")
    bo = nc.dram_tensor("bo", [128, D], F32, kind="ExternalInput")  # pre /2
    # combined causal mask for the diagonal kc-pair: [maskA | maskB]
    maskAB = nc.dram_tensor("maskAB", [128, 2 * QC], F16, kind="ExternalInput")
    out = nc.dram_tensor("out", [S, D], F32, kind="ExternalOutput")

    with tile.TileContext(nc) as tc, ExitStack() as ctx:
        const = ctx.enter_context(tc.tile_pool(name="const", bufs=1))
        psum_proj = ctx.enter_context(
            tc.tile_pool(name="psum_proj", bufs=1, space="PSUM")
        )
        psum_st = ctx.enter_context(tc.tile_pool(name="psum_st", bufs=4, space="PSUM"))
        psum_z = ctx.enter_context(tc.tile_pool(name="psum_z", bufs=3, space="PSUM"))
        pt_pool = ctx.enter_context(tc.tile_pool(name="pt", bufs=4))
        rc_pool = ctx.enter_context(tc.tile_pool(name="rc", bufs=2))
        bc_pool = ctx.enter_context(tc.tile_pool(name="bc", bufs=2))
        zs_pool = ctx.enter_context(tc.tile_pool(name="zs", bufs=2))
        out_pool = ctx.enter_context(tc.tile_pool(name="outp", bufs=3))

        residT_sb = const.tile([128, 6, S], F16)
        posT_sb = const.tile([128, 6, S], F16)
        xqkT_sb = const.tile([128, 6, S], F16)
        wq_sb = const.tile([128, 6, 384], F16)
        wk_sb = const.tile([128, 6, 384], F16)
        wv_sb = const.tile([128, 6, 384], F16)
        wo_sb = const.tile([128, 3, D], F16)
        bq_sb = const.tile([128, NPAIR], F32)
        bk_sb = const.tile([128, NPAIR], F32)
        bv_sb = const.tile([128, 384], F32)
        bo_sb = const.tile([128, D], F32)
        mAB_sb = const.tile([128, 2 * QC], F16)
        qt_sb = const.tile([128, NPAIR, S], F16)
        # K^T pair-stacked like Q^T: pair p slot rows 0:64 = even head,
        # rows 64:128 = odd head (feeds the row-group-packed score matmuls)
        kt_sb = const.tile([128, NPAIR, S], F16)
        v1_sb = const.tile([128, NKC, HG * 65], F16)
        zt2_sb = const.tile([128, NPAIR, S], F16)

        # ---- loads + init ----
        for dc in range(6):
            nc.sync.dma_start(residT_sb[:, dc, :], residT[128 * dc : 128 * dc + 128, :])
            nc.sync.dma_start(posT_sb[:, dc, :], posT[128 * dc : 128 * dc + 128, :])
            nc.sync.dma_start(wq_sb[:, dc, :], wq[128 * dc : 128 * dc + 128, :])
            nc.sync.dma_start(wk_sb[:, dc, :], wk[128 * dc : 128 * dc + 128, :])
            nc.sync.dma_start(wv_sb[:, dc, :], wv[128 * dc : 128 * dc + 128, :])
        for t in range(3):
            nc.sync.dma_start(wo_sb[:, t, :], wo[128 * t : 128 * t + 128, :])
        nc.sync.dma_start(bq_sb[:], bq[:])
        nc.sync.dma_start(bk_sb[:], bk[:])
        nc.sync.dma_start(bv_sb[:], bv[:])
        nc.sync.dma_start(bo_sb[:], bo[:])
        nc.sync.dma_start(mAB_sb[:], maskAB[:])
        nc.gpsimd.memset(v1_sb[:], 1.0)
        # gpsimd ucode library providing InstPartitionBroadcast (softmax
        # denominator broadcast); loaded once, overlaps the input DMAs
        nc.gpsimd.load_library(library_config.attn)

        # ---- phase A: xqk = resid + pos (fp16) ----
        for dc in range(6):
            nc.vector.tensor_add(
                xqkT_sb[:, dc, :], residT_sb[:, dc, :], posT_sb[:, dc, :]
            )

        # ---- phases B-D, optionally repeated for on-hw timing ----
        def _run_phases():
            _phases_bcd(
                nc,
                psum_proj,
                psum_st,
                psum_z,
                psum_bc,
                pt_pool,
                rc_pool,
                bc_pool,
                zs_pool,
                out_pool,
                residT_sb,
                xqkT_sb,
                wq_sb,
                wk_sb,
                wv_sb,
                wo_sb,
                bq_sb,
                bk_sb,
                bv_sb,
                bo_sb,
                mAB_sb,
                qt_sb,
                kt_sb,
                v1_sb,
                zt2_sb,
                ones_sb,
                out,
            )

        if loop_reps is not None:
            with tc.For_i(0, loop_reps, 1):
                _run_phases()
        else:
            for _rep in range(reps):
                _run_phases()

    if split_drains:
        _split_drain_waits(nc)
    return nc


def _phases_bcd(
    nc,
    psum_proj,
    psum_st,
    psum_z,
    psum_bc,
    pt_pool,
    rc_pool,
    bc_pool,
    zs_pool,
    out_pool,
    residT_sb,
    xqkT_sb,
    wq_sb,
    wk_sb,
    wv_sb,
    wo_sb,
    bq_sb,
    bk_sb,
    bv_sb,
    bo_sb,
    mAB_sb,
    qt_sb,
    kt_sb,
    v1_sb,
    zt2_sb,
    ones_sb,
    out,
):
    if True:
        # ---- phase B: projections ----
        for p in range(NPAIR):
            h0, h1 = 2 * p, 2 * p + 1
            for n in range(4):
                sl = slice(512 * n, 512 * n + 512)
                psq = psum_proj.tile([128, 512], F32, tag="proj")
                for dc in range(6):
                    nc.tensor.matmul(
                        psq,
                        lhsT=wq_sb[:, dc, 128 * p : 128 * p + 128],
                        rhs=xqkT_sb[:, dc, sl],
                        start=dc == 0,
                        stop=dc == 5,
                    )
                nc.vector.tensor_scalar(
                    qt_sb[:, p, sl], psq, bq_sb[:, p : p + 1], None, ADD
                )
                psk = psum_proj.tile([128, 512], F32, tag="proj")
                for dc in range(6):
                    nc.tensor.matmul(
                        psk,
                        lhsT=wk_sb[:, dc, 128 * p : 128 * p + 128],
                        rhs=xqkT_sb[:, dc, sl],
                        start=dc == 0,
                        stop=dc == 5,
                    )
                nc.vector.tensor_scalar(
                    kt_sb[:, p, sl], psk, bk_sb[:, p : p + 1], None, ADD
                )

        for n in range(NKC):
            psv = psum_proj.tile([128, 512], F32, tag="proj")
            for dc in range(6):
                nc.tensor.matmul(
                    psv[:, :384],
                    lhsT=residT_sb[:, dc, 128 * n : 128 * n + 128],
                    rhs=wv_sb[:, dc, :],
                    start=dc == 0,
                    stop=dc == 5,
                )
            for h in range(HG):
                nc.vector.tensor_tensor(
                    v1_sb[:, n, 65 * h : 65 * h + 64],
                    psv[:, 64 * h : 64 * h + 64],
                    bv_sb[:, 64 * h : 64 * h + 64],
                    ADD,
                )

        # ---- phase C: attention, head-pair at a time ----
        # Scores for the two heads of a pair run CONCURRENTLY in the PE array:
        # even head's K=64 contraction uses array row-group 0 (partitions
        # 0:64), odd head's uses row-group 2 (partitions 64:128) — the
        # tile_position auto-derives from the lhsT base partition.
        for p in range(NPAIR):
            h0, h1 = 2 * p, 2 * p + 1
            # odd head stages at partitions 0:64; DMA shifts to 64:128 at end
            zstag = zs_pool.tile([64, S], F16, tag="zstag", name="zstag")
            for j in range(NJ):
                qsl = slice(QC * j, QC * j + QC)
                zp0 = psum_z.tile([65, QC], F32, tag="z")
                zp1 = psum_z.tile([65, QC], F32, tag="z")
                for m in range(j + 1):
                    kslA = slice(128 * 2 * m, 128 * 2 * m + 128)
                    kslB = slice(128 * (2 * m + 1), 128 * (2 * m + 1) + 128)
                    sp0 = psum_st.tile([128, 2 * QC], F32, tag="st")
                    sp1 = psum_st.tile([128, 2 * QC], F32, tag="st")
                    for ksl, csl in ((kslA, slice(0, QC)), (kslB, slice(QC, 2 * QC))):
                        nc.tensor.matmul(
                            sp0[:, csl],
                            lhsT=kt_sb[0:64, p, ksl],
                            rhs=qt_sb[0:64, p, qsl],
                            start=True,
                            stop=True,
                        )
                        nc.tensor.matmul(
                            sp1[:, csl],
                            lhsT=kt_sb[64:128, p, ksl],
                            rhs=qt_sb[64:128, p, qsl],
                            start=True,
                            stop=True,
                        )
                    pt0 = pt_pool.tile([128, 2 * QC], F16, tag="pt")
                    pt1 = pt_pool.tile([128, 2 * QC], F16, tag="pt")
                    nc.scalar.activation(pt0, sp0, EXP)
                    nc.scalar.activation(pt1, sp1, EXP)
                    if m == j:  # diagonal kc pair: causal mask
                        nc.vector.tensor_mul(pt0, pt0, mAB_sb)
                        nc.vector.tensor_mul(pt1, pt1, mAB_sb)
                    for kc, csl in ((2 * m, slice(0, QC)), (2 * m + 1, slice(QC, 2 * QC))):
                        nc.tensor.matmul(
                            zp0,
                            lhsT=v1_sb[:, kc, 65 * h0 : 65 * h0 + 65],
                            rhs=pt0[:, csl],
                            start=kc == 0,
                            stop=kc == 2 * j + 1,
                        )
                        nc.tensor.matmul(
                            zp1,
                            lhsT=v1_sb[:, kc, 65 * h1 : 65 * h1 + 65],
                            rhs=pt1[:, csl],
                            start=kc == 0,
                            stop=kc == 2 * j + 1,
                        )
                # normalize: denominator is row 64 of each z psum tile;
                # broadcast 1/denom across 64 partitions via a K=1 PE
                # outer-product with a ones column
                for zp, dst in ((zp0, zt2_sb[0:64, p, qsl]), (zp1, zstag[:, qsl])):
                    rc_t = rc_pool.tile([65, QC], F32, tag="rc", name="rc_t")
                    rc = rc_t[64:65, :]
                    nc.vector.reciprocal(rc, zp[64:65, :])
                    bcp = psum_bc.tile([64, QC], F32, tag="bc", name="bcp")
                    nc.tensor.matmul(
                        bcp, lhsT=ones_sb[64:65, 0:64], rhs=rc, start=True, stop=True
                    )
                    bcs = bc_pool.tile([64, QC], F32, tag="bcs", name="bcs")
                    nc.vector.tensor_copy(bcs, bcp)
                    nc.vector.tensor_tensor(dst, zp[0:64, :], bcs, MULT)
            # partition-shift the odd head into rows 64:128 of the pair slot
            nc.sync.dma_start(zt2_sb[64:128, p, :], zstag[:, :])

        # ---- phase D: output projection ----
        for n in range(NKC):
            ps0 = psum_proj.tile([128, 512], F32, tag="proj")
            ps1 = psum_proj.tile([128, 512], F32, tag="proj")
            for t in range(3):
                nc.tensor.matmul(
                    ps0[:, :384],
                    lhsT=zt2_sb[:, t, 128 * n : 128 * n + 128],
                    rhs=wo_sb[:, t, 0:384],
                    start=t == 0,
                    stop=t == 2,
                )
            for t in range(3):
                nc.tensor.matmul(
                    ps1[:, :384],
                    lhsT=zt2_sb[:, t, 128 * n : 128 * n + 128],
                    rhs=wo_sb[:, t, 384:768],
                    start=t == 0,
                    stop=t == 2,
                )
            ob = out_pool.tile([128, D], F32, tag="out")
            nc.vector.tensor_add(ob[:, 0:384], ps0[:, :384], bo_sb[:, 0:384])
            nc.vector.tensor_add(ob[:, 384:768], ps1[:, :384], bo_sb[:, 384:768])
            nc.sync.dma_start(out[128 * n : 128 * n + 128, :], ob)


def pack_inputs(
    resid_pre, shortformer_pos_embed, W_Q, W_K, W_V, W_O, b_Q, b_K, b_V, b_O
):
    """Build the 8 per-core input maps (host-side shard + transpose + cast)."""
    resid_pre = np.asarray(resid_pre, dtype=np.float32)
    pos = np.asarray(shortformer_pos_embed, dtype=np.float32)
    W_Q = np.asarray(W_Q, dtype=np.float32)
    W_K = np.asarray(W_K, dtype=np.float32)
    W_V = np.asarray(W_V, dtype=np.float32)
    W_O = np.asarray(W_O, dtype=np.float32)
    b_Q = np.asarray(b_Q, dtype=np.float32)
    b_K = np.asarray(b_K, dtype=np.float32)
    b_V = np.asarray(b_V, dtype=np.float32)
    b_O = np.asarray(b_O, dtype=np.float32)

    i_a = np.arange(128)[:, None]
    i_q = np.arange(QC)[None, :]
    maskA = (i_q >= i_a).astype(np.float16)
    maskB = (i_q >= i_a + 128).astype(np.float16)
    maskAB = np.concatenate([maskA, maskB], axis=1)

    in_maps = []
    for c in range(N_CORES):
        b, g = divmod(c, 2)
        hs = slice(HG * g, HG * g + HG)
        wq = np.concatenate([W_Q[h] for h in range(HG * g, HG * g + HG)], axis=1)
        wk = np.concatenate([W_K[h] / 8.0 for h in range(HG * g, HG * g + HG)], axis=1)
        wv = np.concatenate([W_V[h] for h in range(HG * g, HG * g + HG)], axis=1)
        wo = W_O[hs].reshape(HG * Dh, D)
        bqc = b_Q[hs].reshape(NPAIR, 128).T  # [128, 3]; col p = pair p biases
        bkc = (b_K[hs] / 8.0).reshape(NPAIR, 128).T
        bvc = np.broadcast_to(b_V[hs].reshape(1, HG * Dh), (128, HG * Dh))
        boc = np.broadcast_to(b_O[None, :] * 0.5, (128, D))
        in_maps.append(
            {
                "residT": np.ascontiguousarray(resid_pre[b].T).astype(np.float16),
                "posT": np.ascontiguousarray(pos[b].T).astype(np.float16),
                "wq": wq.astype(np.float16),
                "wk": wk.astype(np.float16),
                "wv": wv.astype(np.float16),
                "wo": wo.astype(np.float16),
                "bq": np.ascontiguousarray(bqc),
                "bk": np.ascontiguousarray(bkc),
                "bv": np.ascontiguousarray(bvc),
                "bo": np.ascontiguousarray(boc),
                "maskAB": maskAB,
            }
        )
    return in_maps


class Runner:
    """Compiles the Bass module once (via the bass2jax PJRT path that
    bass_utils.run_bass_kernel_spmd uses under axon) and keeps the jitted
    sharded executable for reuse."""

    def __init__(self, reps=1, loop_reps=None):
        import jax
        from jax.sharding import Mesh, PartitionSpec
        from jax.experimental.shard_map import shard_map
        from concourse import bass2jax

        self.jax = jax
        bass2jax.install_neuronx_cc_hook()
        nc = build_nc(reps=reps, loop_reps=loop_reps)
        self.nc = nc

        partition_name = (
            nc.partition_id_tensor.name if nc.partition_id_tensor else None
        )
        in_names, out_names, out_avals, zero_outs = [], [], [], []
        for alloc in nc.m.functions[0].allocations:
            if not isinstance(alloc, mybir.MemoryLocationSet):
                continue
            name = alloc.memorylocations[0].name
            if alloc.kind == "ExternalInput":
                if name != partition_name:
                    in_names.append(name)
            elif alloc.kind == "ExternalOutput":
                shape = tuple(alloc.tensor_shape)
                dtype = mybir.dt.np(alloc.dtype)
                out_names.append(name)
                out_avals.append(jax.core.ShapedArray(shape, dtype))
                zero_outs.append(np.zeros(shape, dtype))
        self.in_names = list(in_names)
        self.out_names = out_names
        self.out_avals = out_avals
        self.zero_outs = zero_outs
        n_params = len(in_names)
        n_outs = len(out_names)
        all_in_names = in_names + out_names
        if partition_name is not None:
            all_in_names.append(partition_name)

        def _body(*args):
            operands = list(args)
            if partition_name is not None:
                operands.append(bass2jax.partition_id_tensor())
            outs = bass2jax._bass_exec_p.bind(
                *operands,
                out_avals=tuple(out_avals),
                in_names=tuple(all_in_names),
                out_names=tuple(out_names),
                lowering_input_output_aliases=(),
                sim_require_finite=True,
                sim_require_nnan=True,
                nc=nc,
            )
            return tuple(outs)

        devices = jax.devices()[:N_CORES]
        assert len(devices) == N_CORES, f"need {N_CORES} cores, have {len(devices)}"
        mesh = Mesh(np.asarray(devices), ("core",))
        in_specs = (PartitionSpec("core"),) * (n_params + n_outs)
        out_specs = (PartitionSpec("core"),) * n_outs
        donate = tuple(range(n_params, n_params + n_outs))
        self.sharded = jax.jit(
            shard_map(
                _body,
                mesh=mesh,
                in_specs=in_specs,
                out_specs=out_specs,
                check_rep=False,
            ),
            donate_argnums=donate,
            keep_unused=True,
        )

    def concat_inputs(self, in_maps):
        return [
            np.concatenate([np.asarray(m[name]) for m in in_maps], axis=0)
            for name in self.in_names
        ]

    def concat_zeros(self):
        return [
            np.zeros((N_CORES * z.shape[0], *z.shape[1:]), z.dtype)
            for z in self.zero_outs
        ]

    def run_concat(self, concat_in):
        """One sharded execution; returns the concatenated 'out' array
        [8*2048, 768] (per-core blocks along axis 0)."""
        out_arrs = self.sharded(*concat_in, *self.concat_zeros())
        return np.asarray(out_arrs[0])

    def run(self, in_maps):
        flat = self.run_concat(self.concat_inputs(in_maps))
        return flat.reshape(N_CORES, S, D)


_RUNNER = None


def get_runner():
    global _RUNNER
    if _RUNNER is None:
        _RUNNER = Runner()
    return _RUNNER


def kernel(**inputs):
    runner = get_runner()
    in_maps = pack_inputs(**inputs)
    per_core = runner.run(in_maps)
    # host all-reduce of the two head-group partials per batch
    out = per_core[0::2] + per_core[1::2]  # [4, 2048, 768]
    return out.astype(np.float32)


if __name__ == "__main__":
    # quick self-run with random inputs
    rng = np.random.default_rng(0)
    ins = {
        "resid_pre": rng.standard_normal((B, S, D), dtype=np.float32),
        "shortformer_pos_embed": rng.standard_normal((B, S, D), dtype=np.float32),
        "W_Q": rng.standard_normal((H, D, Dh), dtype=np.float32) * 0.02,
        "W_K": rng.standard_normal((H, D, Dh), dtype=np.float32) * 0.02,
        "W_V": rng.standard_normal((H, D, Dh), dtype=np.float32) * 0.02,
        "W_O": rng.standard_normal((H, Dh, D), dtype=np.float32) * 0.02,
        "b_Q": np.zeros((H, Dh), np.float32),
        "b_K": np.zeros((H, Dh), np.float32),
        "b_V": np.zeros((H, Dh), np.float32),
        "b_O": np.zeros((D,), np.float32),
    }
    out = kernel(**ins)
    print("out", out.shape, out.dtype, float(np.abs(out).max()))


# revision 45
# speedup vs baseline: 191.1018x; 1.1132x over previous
"""Trainium2 Bass kernel for nn_Attention_49478023250074.

Multi-head causal attention (shortformer positional embedding variant):
  B=4, S=2048, D=768, H=12, Dh=64.

Sharding: 8 NeuronCores = data-parallel over batch (4) x tensor-parallel over
heads (2 groups of 6). Each core computes the full attention output
contribution of its 6 heads for its batch; the output projection partial sums
are reduced on the host (the "all-reduce" of the sharding hint).

On-core dataflow (matmul operands fp16, accumulation fp32):
  inputs arrive pre-transposed d-major: residT, xqkT = (resid+pos).T  [768,S]
  QT/KT[pair] = (W2.T @ x).T-layout [e2=128, S]      (PE, per 2-head pair;
                                                      K pre-scaled by 1/8)
  V1 = [V_h | ones] blocks [S, 6*65]                 (ones column -> denominator)
  per (head-pair, 256-wide q-chunk j, kc-pair m):
     4 score matmuls K=64 -> one 2-bank PSUM tile    (the two heads run
       [h0@kcA | h0@kcB | h1@kcA | h1@kcB]            CONCURRENTLY in PE row
                                                      groups 0 / 2)
     P = exp(tile) in ONE [128,1024] ACT op; diagonal kc-pair masked by a
       single doubled 0/1 mask multiply              (ACT + DVE)
     z chains: zp[65, q] += V1[kc].T @ P-quarter     (row 64 = denominator)
     normalize: 1/denom -> K=1 ones outer-product into the z tile's spare
       columns -> SBUF copy -> one multiply; odd head DMA-shifted into the
       pair-stacked zt2 layout (partitions 64:128)
  out = zt2.T @ Wo + b_O/2                           (PE + DVE, per q-chunk)
"""

import os
import sys

for _p in ("/opt/trn_rl_repo", "/root/.axon_site/_ro/trn_rl_repo"):
    if os.path.isdir(_p) and _p not in sys.path:
        sys.path.insert(0, _p)

from contextlib import ExitStack

import numpy as np

import concourse.bass as bass
import concourse.mybir as mybir
import concourse.tile as tile
from concourse import library_config

B, S, D, H, Dh = 4, 2048, 768, 12, 64
HG = 6          # heads per core
NPAIR = 3       # head pairs per core
QC = 256        # attention q-chunk width
NJ = S // QC    # q-chunks
NKC = S // 128  # k-chunks
F16 = mybir.dt.float16
F32 = mybir.dt.float32
ADD = mybir.AluOpType.add
MULT = mybir.AluOpType.mult
EXP = mybir.ActivationFunctionType.Exp
N_CORES = 8


def _split_drain_waits(nc):
    """The walrus build in this environment accepts only ONE sync-wait per
    instruction (setupSyncWait raises 'Too many sync wait commands' for two or
    more). Peel extra waits onto no-ops placed just before the instruction on
    the same engine — same-engine program order keeps the semantics: the
    engine stalls on the peeled waits before reaching the instruction."""
    for f in nc.m.functions:
        for bb in f.blocks:
            insts = list(bb.instructions)
            out, changed = [], False
            for ins in insts:
                si = ins.sync_info
                if si is not None and len(si.on_wait) > 1:
                    waits = list(si.on_wait)
                    for i, w in enumerate(waits[:-1]):
                        out.append(
                            mybir.InstNoOp(
                                name=f"{ins.name}-w{i}",
                                engine=ins.engine,
                                sync_info=mybir.SyncInfo(on_wait=[w], on_update=[]),
                                bass_nofuse=True,
                            )
                        )
                    ins.sync_info = mybir.SyncInfo(
                        on_wait=[waits[-1]], on_update=list(si.on_update)
                    )
                    changed = True
                out.append(ins)
            if changed:
                bb.instructions.clear()
                for i2 in out:
                    bb.add_instruction(i2)


def build_nc(split_drains=True, reps=1, loop_reps=None):
    """loop_reps: wrap the compute phases in a tc.For_i hardware loop (for
    on-device timing: wall(loop_reps=N) - wall(loop_reps=None) ~= (N-1)*T)."""
    nc = bass.Bass()
    residT = nc.dram_tensor("residT", [D, S], F16, kind="ExternalInput")
    xqkT = nc.dram_tensor("xqkT", [D, S], F16, kind="ExternalInput")
    wq = nc.dram_tensor("wq", [D, 384], F16, kind="ExternalInput")
    wk = nc.dram_tensor("wk", [D, 384], F16, kind="ExternalInput")  # pre /8
    wv = nc.dram_tensor("wv", [D, 384], F16, kind="ExternalInput")
    wo = nc.dram_tensor("wo", [384, D], F16, kind="ExternalInput")
    bq = nc.dram_tensor("bq", [128, NPAIR], F32, kind="ExternalInput")
    bk = nc.dram_tensor("bk", [128, NPAIR], F32, kind="ExternalInput")  # pre /8
    bv = nc.dram_tensor("bv", [128, 384], F32, kind="ExternalInput")
    bo = nc.dram_tensor("bo", [128, D], F32, kind="ExternalInput")  # pre /2
    # causal mask for the diagonal kc-pair, doubled for the two heads that
    # share one score tile: [maskA | maskB | maskA | maskB]
    maskAB = nc.dram_tensor("maskAB", [128, 4 * QC], F16, kind="ExternalInput")
    out = nc.dram_tensor("out", [S, D], F32, kind="ExternalOutput")

    with tile.TileContext(nc) as tc, ExitStack() as ctx:
        const = ctx.enter_context(tc.tile_pool(name="const", bufs=1))
        psum_st = ctx.enter_context(tc.tile_pool(name="psum_st", bufs=2, space="PSUM"))
        psum_z = ctx.enter_context(tc.tile_pool(name="psum_z", bufs=4, space="PSUM"))
        pt_pool = ctx.enter_context(tc.tile_pool(name="pt", bufs=6))
        rc_pool = ctx.enter_context(tc.tile_pool(name="rc", bufs=2))
        bc_pool = ctx.enter_context(tc.tile_pool(name="bc", bufs=2))
        zs_pool = ctx.enter_context(tc.tile_pool(name="zs", bufs=2))
        out_pool = ctx.enter_context(tc.tile_pool(name="outp", bufs=3))

        residT_sb = const.tile([128, 6, S], F16)
        xqkT_sb = const.tile([128, 6, S], F16)
        wq_sb = const.tile([128, 6, 384], F16)
        wk_sb = const.tile([128, 6, 384], F16)
        wv_sb = const.tile([128, 6, 384], F16)
        wo_sb = const.tile([128, 3, D], F16)
        bq_sb = const.tile([128, NPAIR], F32)
        bk_sb = const.tile([128, NPAIR], F32)
        bv_sb = const.tile([128, 384], F32)
        bo_sb = const.tile([128, D], F32)
        mAB_sb = const.tile([128, 4 * QC], F16)
        qt_sb = const.tile([128, NPAIR, S], F16)
        # K^T pair-stacked like Q^T: pair p slot rows 0:64 = even head,
        # rows 64:128 = odd head (feeds the row-group-packed score matmuls)
        kt_sb = const.tile([128, NPAIR, S], F16)
        v1_sb = const.tile([128, NKC, HG * 65], F16)
        zt2_sb = const.tile([128, NPAIR, S], F16)
        # ones row at partition 64: the K=1 denominator-broadcast matmul's
        # operands must share the denominator row's partition window
        ones_sb = const.tile([65, 128], F32)

        # ---- loads + init ----
        for dc in range(6):
            nc.sync.dma_start(residT_sb[:, dc, :], residT[128 * dc : 128 * dc + 128, :])
            nc.sync.dma_start(xqkT_sb[:, dc, :], xqkT[128 * dc : 128 * dc + 128, :])
            nc.sync.dma_start(wq_sb[:, dc, :], wq[128 * dc : 128 * dc + 128, :])
            nc.sync.dma_start(wk_sb[:, dc, :], wk[128 * dc : 128 * dc + 128, :])
            nc.sync.dma_start(wv_sb[:, dc, :], wv[128 * dc : 128 * dc + 128, :])
        for t in range(3):
            nc.sync.dma_start(wo_sb[:, t, :], wo[128 * t : 128 * t + 128, :])
        nc.sync.dma_start(bq_sb[:], bq[:])
        nc.sync.dma_start(bk_sb[:], bk[:])
        nc.sync.dma_start(bv_sb[:], bv[:])
        nc.sync.dma_start(bo_sb[:], bo[:])
        nc.sync.dma_start(mAB_sb[:], maskAB[:])
        nc.gpsimd.memset(v1_sb[:], 1.0)
        nc.gpsimd.memset(ones_sb[:], 1.0)

        # ---- phases B-D, optionally repeated for on-hw timing ----
        def _run_phases():
            _phases_bcd(
                nc,
                psum_st,
                psum_z,
                pt_pool,
                rc_pool,
                bc_pool,
                zs_pool,
                out_pool,
                residT_sb,
                xqkT_sb,
                wq_sb,
                wk_sb,
                wv_sb,
                wo_sb,
                bq_sb,
                bk_sb,
                bv_sb,
                bo_sb,
                mAB_sb,
                qt_sb,
                kt_sb,
                v1_sb,
                zt2_sb,
                ones_sb,
                out,
            )

        if loop_reps is not None:
            with tc.For_i(0, loop_reps, 1):
                _run_phases()
        else:
            for _rep in range(reps):
                _run_phases()

    if split_drains:
        _split_drain_waits(nc)
    return nc


def _phases_bcd(
    nc,
    psum_st,
    psum_z,
    pt_pool,
    rc_pool,
    bc_pool,
    zs_pool,
    out_pool,
    residT_sb,
    xqkT_sb,
    wq_sb,
    wk_sb,
    wv_sb,
    wo_sb,
    bq_sb,
    bk_sb,
    bv_sb,
    bo_sb,
    mAB_sb,
    qt_sb,
    kt_sb,
    v1_sb,
    zt2_sb,
    ones_sb,
    out,
):
    if True:
        # ---- phase B: projections (QK of pair 0 first so attention on pair
        # 0 can start while pairs 1-2 still project; psum from the deep
        # st pool so accumulation chains double-buffer) ----
        def _proj_qk(p):
            for n in range(4):
                sl = slice(512 * n, 512 * n + 512)
                psq = psum_st.tile([128, 512], F32, tag="st", name="psq")
                for dc in range(6):
                    nc.tensor.matmul(
                        psq,
                        lhsT=wq_sb[:, dc, 128 * p : 128 * p + 128],
                        rhs=xqkT_sb[:, dc, sl],
                        start=dc == 0,
                        stop=dc == 5,
                    )
                nc.vector.tensor_scalar(
                    qt_sb[:, p, sl], psq, bq_sb[:, p : p + 1], None, ADD
                )
                psk = psum_st.tile([128, 512], F32, tag="st", name="psk")
                for dc in range(6):
                    nc.tensor.matmul(
                        psk,
                        lhsT=wk_sb[:, dc, 128 * p : 128 * p + 128],
                        rhs=xqkT_sb[:, dc, sl],
                        start=dc == 0,
                        stop=dc == 5,
                    )
                nc.vector.tensor_scalar(
                    kt_sb[:, p, sl], psk, bk_sb[:, p : p + 1], None, ADD
                )

        _proj_qk(0)
        for n in range(NKC):
            psv = psum_st.tile([128, 512], F32, tag="st", name="psv")
            for dc in range(6):
                nc.tensor.matmul(
                    psv[:, :384],
                    lhsT=residT_sb[:, dc, 128 * n : 128 * n + 128],
                    rhs=wv_sb[:, dc, :],
                    start=dc == 0,
                    stop=dc == 5,
                )
            for h in range(HG):
                nc.vector.tensor_tensor(
                    v1_sb[:, n, 65 * h : 65 * h + 64],
                    psv[:, 64 * h : 64 * h + 64],
                    bv_sb[:, 64 * h : 64 * h + 64],
                    ADD,
                )
        _proj_qk(1)
        _proj_qk(2)

        # ---- phase C: attention, pair-outer. Scores for the two heads of a
        # pair run CONCURRENTLY in the PE array: even head's K=64 contraction
        # uses array row-group 0 (partitions 0:64), odd head's row-group 2
        # (partitions 64:128) — tile_position auto-derives from the lhsT
        # base partition. ----
        for p in range(NPAIR):
            h0, h1 = 2 * p, 2 * p + 1
            for j in range(NJ):
                qsl = slice(QC * j, QC * j + QC)
                # cols 0:QC = z accumulation; cols QC:2QC = 1/denom bcast
                zp0 = psum_z.tile([65, 2 * QC], F32, tag="z", name="zp0")
                zp1 = psum_z.tile([65, 2 * QC], F32, tag="z", name="zp1")
                for m in range(j + 1):
                    kslA = slice(128 * 2 * m, 128 * 2 * m + 128)
                    kslB = slice(128 * (2 * m + 1), 128 * (2 * m + 1) + 128)
                    # one 2-bank tile holds both heads' score pair:
                    # cols [0:QC | QC:2QC | 2QC:3QC | 3QC:4QC] =
                    #      [h0@kcA | h0@kcB | h1@kcA | h1@kcB]
                    sp = psum_st.tile([128, 4 * QC], F32, tag="st", name="sp")
                    for ksl, c0 in ((kslA, 0), (kslB, QC)):
                        nc.tensor.matmul(
                            sp[:, c0 : c0 + QC],
                            lhsT=kt_sb[0:64, p, ksl],
                            rhs=qt_sb[0:64, p, qsl],
                            start=True,
                            stop=True,
                        )
                        nc.tensor.matmul(
                            sp[:, 2 * QC + c0 : 2 * QC + c0 + QC],
                            lhsT=kt_sb[64:128, p, ksl],
                            rhs=qt_sb[64:128, p, qsl],
                            start=True,
                            stop=True,
                        )
                    pt = pt_pool.tile([128, 4 * QC], F16, tag="pt", name="pt")
                    nc.scalar.activation(pt, sp, EXP)
                    if m == j:  # diagonal kc pair: causal mask (both heads)
                        nc.vector.tensor_mul(pt, pt, mAB_sb)
                    for kc, csl0, csl1 in (
                        (2 * m, slice(0, QC), slice(2 * QC, 3 * QC)),
                        (2 * m + 1, slice(QC, 2 * QC), slice(3 * QC, 4 * QC)),
                    ):
                        nc.tensor.matmul(
                            zp0[:, 0:QC],
                            lhsT=v1_sb[:, kc, 65 * h0 : 65 * h0 + 65],
                            rhs=pt[:, csl0],
                            start=kc == 0,
                            stop=kc == 2 * j + 1,
                        )
                        nc.tensor.matmul(
                            zp1[:, 0:QC],
                            lhsT=v1_sb[:, kc, 65 * h1 : 65 * h1 + 65],
                            rhs=pt[:, csl1],
                            start=kc == 0,
                            stop=kc == 2 * j + 1,
                        )
                # normalize: denominator is row 64 (cols 0:QC) of each z
                # psum tile; a K=1 ones outer-product broadcasts 1/denom
                # into the tile's spare columns, a DVE copy moves it to SBUF
                # (DVE has a single PSUM read port), then one multiply
                for zp, slot in ((zp0, 0), (zp1, 1)):
                    rc_t = rc_pool.tile([65, QC], F32, tag="rc", name="rc_t")
                    rc = rc_t[64:65, :]
                    nc.vector.reciprocal(rc, zp[64:65, 0:QC])
                    nc.tensor.matmul(
                        zp[0:64, QC : 2 * QC],
                        lhsT=ones_sb[64:65, 0:64],
                        rhs=rc,
                        start=True,
                        stop=True,
                    )
                    bc = bc_pool.tile([64, QC], F32, tag="bc", name="bc")
                    nc.vector.tensor_copy(bc, zp[0:64, QC : 2 * QC])
                    if slot == 0:
                        nc.vector.tensor_tensor(
                            zt2_sb[0:64, p, qsl], zp[0:64, 0:QC], bc, MULT
                        )
                    else:
                        # odd head: stage at partitions 0:64, DMA-shift into
                        # rows 64:128 of the pair slot
                        zs = zs_pool.tile([64, QC], F16, tag="zs", name="zs")
                        nc.vector.tensor_tensor(zs, zp[0:64, 0:QC], bc, MULT)
                        nc.sync.dma_start(zt2_sb[64:128, p, qsl], zs)

        # ---- phase D: output projection (after all pairs) ----
        for n in range(NKC):
            ps0 = psum_st.tile([128, 512], F32, tag="st", name="ps0")
            for t in range(3):
                nc.tensor.matmul(
                    ps0[:, :384],
                    lhsT=zt2_sb[:, t, 128 * n : 128 * n + 128],
                    rhs=wo_sb[:, t, 0:384],
                    start=t == 0,
                    stop=t == 2,
                )
            ob = out_pool.tile([128, D], F32, tag="out", name="ob")
            nc.vector.tensor_add(ob[:, 0:384], ps0[:, :384], bo_sb[:, 0:384])
            ps1 = psum_st.tile([128, 512], F32, tag="st", name="ps1")
            for t in range(3):
                nc.tensor.matmul(
                    ps1[:, :384],
                    lhsT=zt2_sb[:, t, 128 * n : 128 * n + 128],
                    rhs=wo_sb[:, t, 384:768],
                    start=t == 0,
                    stop=t == 2,
                )
            nc.vector.tensor_add(ob[:, 384:768], ps1[:, :384], bo_sb[:, 384:768])
            nc.sync.dma_start(out[128 * n : 128 * n + 128, :], ob)


def pack_inputs(
    resid_pre, shortformer_pos_embed, W_Q, W_K, W_V, W_O, b_Q, b_K, b_V, b_O
):
    """Build the 8 per-core input maps (host-side shard + transpose + cast)."""
    resid_pre = np.asarray(resid_pre, dtype=np.float32)
    pos = np.asarray(shortformer_pos_embed, dtype=np.float32)
    W_Q = np.asarray(W_Q, dtype=np.float32)
    W_K = np.asarray(W_K, dtype=np.float32)
    W_V = np.asarray(W_V, dtype=np.float32)
    W_O = np.asarray(W_O, dtype=np.float32)
    b_Q = np.asarray(b_Q, dtype=np.float32)
    b_K = np.asarray(b_K, dtype=np.float32)
    b_V = np.asarray(b_V, dtype=np.float32)
    b_O = np.asarray(b_O, dtype=np.float32)

    i_a = np.arange(128)[:, None]
    i_q = np.arange(QC)[None, :]
    maskA = (i_q >= i_a).astype(np.float16)
    maskB = (i_q >= i_a + 128).astype(np.float16)
    maskAB = np.concatenate([maskA, maskB, maskA, maskB], axis=1)

    # per-batch tensors (shared by the two head-group cores of each batch)
    residT_b = [
        np.ascontiguousarray(resid_pre[b].T).astype(np.float16) for b in range(B)
    ]
    xqkT_b = [
        np.ascontiguousarray((resid_pre[b] + pos[b]).T).astype(np.float16)
        for b in range(B)
    ]
    # per head-group tensors (shared by the four batch cores of each group)
    grp = []
    for g in range(2):
        hs = slice(HG * g, HG * g + HG)
        wq = np.concatenate([W_Q[h] for h in range(HG * g, HG * g + HG)], axis=1)
        wk = np.concatenate([W_K[h] / 8.0 for h in range(HG * g, HG * g + HG)], axis=1)
        wv = np.concatenate([W_V[h] for h in range(HG * g, HG * g + HG)], axis=1)
        wo = W_O[hs].reshape(HG * Dh, D)
        bqc = b_Q[hs].reshape(NPAIR, 128).T  # [128, 3]; col p = pair p biases
        bkc = (b_K[hs] / 8.0).reshape(NPAIR, 128).T
        bvc = np.broadcast_to(b_V[hs].reshape(1, HG * Dh), (128, HG * Dh))
        boc = np.broadcast_to(b_O[None, :] * 0.5, (128, D))
        grp.append(
            {
                "wq": wq.astype(np.float16),
                "wk": wk.astype(np.float16),
                "wv": wv.astype(np.float16),
                "wo": wo.astype(np.float16),
                "bq": np.ascontiguousarray(bqc),
                "bk": np.ascontiguousarray(bkc),
                "bv": np.ascontiguousarray(bvc),
                "bo": np.ascontiguousarray(boc),
                "maskAB": maskAB,
            }
        )
    in_maps = []
    for c in range(N_CORES):
        b, g = divmod(c, 2)
        in_maps.append({"residT": residT_b[b], "xqkT": xqkT_b[b], **grp[g]})
    return in_maps


class Runner:
    """Compiles the Bass module once (via the bass2jax PJRT path that
    bass_utils.run_bass_kernel_spmd uses under axon) and keeps the jitted
    sharded executable for reuse."""

    def __init__(self, reps=1, loop_reps=None):
        import jax
        from jax.sharding import Mesh, PartitionSpec
        from jax.experimental.shard_map import shard_map
        from concourse import bass2jax

        self.jax = jax
        bass2jax.install_neuronx_cc_hook()
        nc = build_nc(reps=reps, loop_reps=loop_reps)
        self.nc = nc

        partition_name = (
            nc.partition_id_tensor.name if nc.partition_id_tensor else None
        )
        in_names, out_names, out_avals, zero_outs = [], [], [], []
        for alloc in nc.m.functions[0].allocations:
            if not isinstance(alloc, mybir.MemoryLocationSet):
                continue
            name = alloc.memorylocations[0].name
            if alloc.kind == "ExternalInput":
                if name != partition_name:
                    in_names.append(name)
            elif alloc.kind == "ExternalOutput":
                shape = tuple(alloc.tensor_shape)
                dtype = mybir.dt.np(alloc.dtype)
                out_names.append(name)
                out_avals.append(jax.core.ShapedArray(shape, dtype))
                zero_outs.append(np.zeros(shape, dtype))
        self.in_names = list(in_names)
        self.out_names = out_names
        self.out_avals = out_avals
        self.zero_outs = zero_outs
        n_params = len(in_names)
        n_outs = len(out_names)
        all_in_names = in_names + out_names
        if partition_name is not None:
            all_in_names.append(partition_name)

        def _body(*args):
            operands = list(args)
            if partition_name is not None:
                operands.append(bass2jax.partition_id_tensor())
            outs = bass2jax._bass_exec_p.bind(
                *operands,
                out_avals=tuple(out_avals),
                in_names=tuple(all_in_names),
                out_names=tuple(out_names),
                lowering_input_output_aliases=(),
                sim_require_finite=True,
                sim_require_nnan=True,
                nc=nc,
            )
            return tuple(outs)

        devices = jax.devices()[:N_CORES]
        assert len(devices) == N_CORES, f"need {N_CORES} cores, have {len(devices)}"
        mesh = Mesh(np.asarray(devices), ("core",))
        in_specs = (PartitionSpec("core"),) * (n_params + n_outs)
        out_specs = (PartitionSpec("core"),) * n_outs
        donate = tuple(range(n_params, n_params + n_outs))
        self.sharded = jax.jit(
            shard_map(
                _body,
                mesh=mesh,
                in_specs=in_specs,
                out_specs=out_specs,
                check_rep=False,
            ),
            donate_argnums=donate,
            keep_unused=True,
        )

    def concat_inputs(self, in_maps):
        return [
            np.concatenate([np.asarray(m[name]) for m in in_maps], axis=0)
            for name in self.in_names
        ]

    def concat_zeros(self):
        return [
            np.zeros((N_CORES * z.shape[0], *z.shape[1:]), z.dtype)
            for z in self.zero_outs
        ]

    def run_concat(self, concat_in):
        """One sharded execution; returns the concatenated 'out' array
        [8*2048, 768] (per-core blocks along axis 0)."""
        out_arrs = self.sharded(*concat_in, *self.concat_zeros())
        return np.asarray(out_arrs[0])

    def run(self, in_maps):
        flat = self.run_concat(self.concat_inputs(in_maps))
        return flat.reshape(N_CORES, S, D)


_RUNNER = None


def get_runner():
    global _RUNNER
    if _RUNNER is None:
        _RUNNER = Runner()
    return _RUNNER


def kernel(**inputs):
    runner = get_runner()
    in_maps = pack_inputs(**inputs)
    per_core = runner.run(in_maps)
    # host all-reduce of the two head-group partials per batch
    out = per_core[0::2] + per_core[1::2]  # [4, 2048, 768]
    return out.astype(np.float32)


if __name__ == "__main__":
    # quick self-run with random inputs
    rng = np.random.default_rng(0)
    ins = {
        "resid_pre": rng.standard_normal((B, S, D), dtype=np.float32),
        "shortformer_pos_embed": rng.standard_normal((B, S, D), dtype=np.float32),
        "W_Q": rng.standard_normal((H, D, Dh), dtype=np.float32) * 0.02,
        "W_K": rng.standard_normal((H, D, Dh), dtype=np.float32) * 0.02,
        "W_V": rng.standard_normal((H, D, Dh), dtype=np.float32) * 0.02,
        "W_O": rng.standard_normal((H, Dh, D), dtype=np.float32) * 0.02,
        "b_Q": np.zeros((H, Dh), np.float32),
        "b_K": np.zeros((H, Dh), np.float32),
        "b_V": np.zeros((H, Dh), np.float32),
        "b_O": np.zeros((D,), np.float32),
    }
    out = kernel(**ins)
    print("out", out.shape, out.dtype, float(np.abs(out).max()))


# revision 51
# speedup vs baseline: 205.8875x; 1.0774x over previous
"""Trainium2 Bass kernel for nn_Attention_49478023250074.

Multi-head causal attention (shortformer positional embedding variant):
  B=4, S=2048, D=768, H=12, Dh=64.

Sharding: 8 NeuronCores = data-parallel over batch (4) x tensor-parallel over
heads (2 groups of 6). Each core computes the full attention output
contribution of its 6 heads for its batch; the output projection partial sums
are reduced on the host (the "all-reduce" of the sharding hint).

On-core dataflow (matmul operands fp16, accumulation fp32):
  inputs arrive pre-transposed d-major: residT, xqkT = (resid+pos).T  [768,S]
  QT/KT[pair] = (W2.T @ x).T-layout [e2=128, S]      (PE, per 2-head pair;
                                                      K pre-scaled by 1/8)
  V1 = [V_h | ones] blocks [S, 6*65]                 (ones column -> denominator)
  per (head-pair, 256-wide q-chunk j, kc-pair m):
     4 score matmuls K=64 -> one 2-bank PSUM tile    (the two heads run
       [h0@kcA | h0@kcB | h1@kcA | h1@kcB]            CONCURRENTLY in PE row
                                                      groups 0 / 2)
     P = exp(tile) in ONE [128,1024] ACT op; diagonal kc-pair masked by a
       single doubled 0/1 mask multiply              (ACT + DVE)
     z chains: zp[65, q] += V1[kc].T @ P-quarter     (row 64 = denominator)
     normalize: 1/denom -> K=1 ones outer-product into the z tile's spare
       columns -> SBUF copy -> one multiply; odd head DMA-shifted into the
       pair-stacked zt2 layout (partitions 64:128)
  out = zt2.T @ Wo + b_O/2                           (PE + DVE, per q-chunk)
"""

import os
import sys

for _p in ("/opt/trn_rl_repo", "/root/.axon_site/_ro/trn_rl_repo"):
    if os.path.isdir(_p) and _p not in sys.path:
        sys.path.insert(0, _p)

from contextlib import ExitStack

import numpy as np

import concourse.bass as bass
import concourse.mybir as mybir
import concourse.tile as tile

B, S, D, H, Dh = 4, 2048, 768, 12, 64
HG = 6          # heads per core
NPAIR = 3       # head pairs per core
QC = 256        # attention q-chunk width
NJ = S // QC    # q-chunks
NKC = S // 128  # k-chunks
F16 = mybir.dt.float16
F32 = mybir.dt.float32
ADD = mybir.AluOpType.add
MULT = mybir.AluOpType.mult
EXP = mybir.ActivationFunctionType.Exp
N_CORES = 8


def _split_drain_waits(nc):
    """The walrus build in this environment accepts only ONE sync-wait per
    instruction (setupSyncWait raises 'Too many sync wait commands' for two or
    more). Peel extra waits onto no-ops placed just before the instruction on
    the same engine — same-engine program order keeps the semantics: the
    engine stalls on the peeled waits before reaching the instruction."""
    for f in nc.m.functions:
        for bb in f.blocks:
            insts = list(bb.instructions)
            out, changed = [], False
            for ins in insts:
                si = ins.sync_info
                if si is not None and len(si.on_wait) > 1:
                    waits = list(si.on_wait)
                    for i, w in enumerate(waits[:-1]):
                        out.append(
                            mybir.InstNoOp(
                                name=f"{ins.name}-w{i}",
                                engine=ins.engine,
                                sync_info=mybir.SyncInfo(on_wait=[w], on_update=[]),
                                bass_nofuse=True,
                            )
                        )
                    ins.sync_info = mybir.SyncInfo(
                        on_wait=[waits[-1]], on_update=list(si.on_update)
                    )
                    changed = True
                out.append(ins)
            if changed:
                bb.instructions.clear()
                for i2 in out:
                    bb.add_instruction(i2)


def build_nc(split_drains=True, reps=1, loop_reps=None, phases='bcd'):
    """loop_reps: wrap the compute phases in a tc.For_i hardware loop (for
    on-device timing: wall(loop_reps=N) - wall(loop_reps=None) ~= (N-1)*T)."""
    nc = bass.Bass()
    residT = nc.dram_tensor("residT", [D, S], F16, kind="ExternalInput")
    xqkT = nc.dram_tensor("xqkT", [D, S], F16, kind="ExternalInput")
    wq = nc.dram_tensor("wq", [D, 384], F16, kind="ExternalInput")
    wk = nc.dram_tensor("wk", [D, 384], F16, kind="ExternalInput")  # pre /8
    wv = nc.dram_tensor("wv", [D, 384], F16, kind="ExternalInput")
    wo = nc.dram_tensor("wo", [384, D], F16, kind="ExternalInput")
    bq = nc.dram_tensor("bq", [128, NPAIR], F32, kind="ExternalInput")
    bk = nc.dram_tensor("bk", [128, NPAIR], F32, kind="ExternalInput")  # pre /8
    bv = nc.dram_tensor("bv", [128, 384], F32, kind="ExternalInput")
    bo = nc.dram_tensor("bo", [128, D], F32, kind="ExternalInput")  # pre /2
    # causal mask for the diagonal kc-pair, doubled for the two heads that
    # share one score tile: [maskA | maskB | maskA | maskB]
    maskAB = nc.dram_tensor("maskAB", [128, 4 * QC], F16, kind="ExternalInput")
    out = nc.dram_tensor("out", [S, D], F32, kind="ExternalOutput")

    with tile.TileContext(nc) as tc, ExitStack() as ctx:
        const = ctx.enter_context(tc.tile_pool(name="const", bufs=1))
        psum_st = ctx.enter_context(tc.tile_pool(name="psum_st", bufs=2, space="PSUM"))
        psum_z = ctx.enter_context(tc.tile_pool(name="psum_z", bufs=4, space="PSUM"))
        pt_pool = ctx.enter_context(tc.tile_pool(name="pt", bufs=6))
        rc_pool = ctx.enter_context(tc.tile_pool(name="rc", bufs=3))
        bc_pool = ctx.enter_context(tc.tile_pool(name="bc", bufs=3))
        zs_pool = ctx.enter_context(tc.tile_pool(name="zs", bufs=3))
        out_pool = ctx.enter_context(tc.tile_pool(name="outp", bufs=3))

        residT_sb = const.tile([128, 6, S], F16)
        xqkT_sb = const.tile([128, 6, S], F16)
        wq_sb = const.tile([128, 6, 384], F16)
        wk_sb = const.tile([128, 6, 384], F16)
        wv_sb = const.tile([128, 6, 384], F16)
        wo_sb = const.tile([128, 3, D], F16)
        bq_sb = const.tile([128, NPAIR], F32)
        bk_sb = const.tile([128, NPAIR], F32)
        bv_sb = const.tile([128, 384], F32)
        bo_sb = const.tile([128, D], F32)
        mAB_sb = const.tile([128, 4 * QC], F16)
        qt_sb = const.tile([128, NPAIR, S], F16)
        # K^T pair-stacked like Q^T: pair p slot rows 0:64 = even head,
        # rows 64:128 = odd head (feeds the row-group-packed score matmuls)
        kt_sb = const.tile([128, NPAIR, S], F16)
        v1_sb = const.tile([128, NKC, HG * 65], F16)
        zt2_sb = const.tile([128, NPAIR, S], F16)
        # ones row at partition 64: the K=1 denominator-broadcast matmul's
        # operands must share the denominator row's partition window
        ones_sb = const.tile([65, 128], F32)

        # ---- loads + init ----
        for dc in range(6):
            nc.sync.dma_start(residT_sb[:, dc, :], residT[128 * dc : 128 * dc + 128, :])
            nc.sync.dma_start(xqkT_sb[:, dc, :], xqkT[128 * dc : 128 * dc + 128, :])
            nc.sync.dma_start(wq_sb[:, dc, :], wq[128 * dc : 128 * dc + 128, :])
            nc.sync.dma_start(wk_sb[:, dc, :], wk[128 * dc : 128 * dc + 128, :])
            nc.sync.dma_start(wv_sb[:, dc, :], wv[128 * dc : 128 * dc + 128, :])
        for t in range(3):
            nc.sync.dma_start(wo_sb[:, t, :], wo[128 * t : 128 * t + 128, :])
        nc.sync.dma_start(bq_sb[:], bq[:])
        nc.sync.dma_start(bk_sb[:], bk[:])
        nc.sync.dma_start(bv_sb[:], bv[:])
        nc.sync.dma_start(bo_sb[:], bo[:])
        nc.sync.dma_start(mAB_sb[:], maskAB[:])
        nc.gpsimd.memset(v1_sb[:], 1.0)
        nc.gpsimd.memset(ones_sb[:], 1.0)

        # ---- phases B-D, optionally repeated for on-hw timing ----
        def _run_phases():
            _phases_bcd(
                phases,
                nc,
                psum_st,
                psum_z,
                pt_pool,
                rc_pool,
                bc_pool,
                zs_pool,
                out_pool,
                residT_sb,
                xqkT_sb,
                wq_sb,
                wk_sb,
                wv_sb,
                wo_sb,
                bq_sb,
                bk_sb,
                bv_sb,
                bo_sb,
                mAB_sb,
                qt_sb,
                kt_sb,
                v1_sb,
                zt2_sb,
                ones_sb,
                out,
            )

        if loop_reps is not None:
            with tc.For_i(0, loop_reps, 1):
                _run_phases()
        else:
            for _rep in range(reps):
                _run_phases()

    if split_drains:
        _split_drain_waits(nc)
    return nc


def _phases_bcd(
    phases,
    nc,
    psum_st,
    psum_z,
    pt_pool,
    rc_pool,
    bc_pool,
    zs_pool,
    out_pool,
    residT_sb,
    xqkT_sb,
    wq_sb,
    wk_sb,
    wv_sb,
    wo_sb,
    bq_sb,
    bk_sb,
    bv_sb,
    bo_sb,
    mAB_sb,
    qt_sb,
    kt_sb,
    v1_sb,
    zt2_sb,
    ones_sb,
    out,
):
    if True:
        # ---- phase B: projections (QK of pair 0 first so attention on pair
        # 0 can start while pairs 1-2 still project; psum from the deep
        # st pool so accumulation chains double-buffer) ----
        def _proj_qk(p):
            for n in range(4):
                sl = slice(512 * n, 512 * n + 512)
                psq = psum_z.tile([128, 512], F32, tag="z", name="psq")
                for dc in range(6):
                    nc.tensor.matmul(
                        psq,
                        lhsT=wq_sb[:, dc, 128 * p : 128 * p + 128],
                        rhs=xqkT_sb[:, dc, sl],
                        start=dc == 0,
                        stop=dc == 5,
                    )
                nc.vector.tensor_scalar(
                    qt_sb[:, p, sl], psq, bq_sb[:, p : p + 1], None, ADD
                )
                psk = psum_z.tile([128, 512], F32, tag="z", name="psk")
                for dc in range(6):
                    nc.tensor.matmul(
                        psk,
                        lhsT=wk_sb[:, dc, 128 * p : 128 * p + 128],
                        rhs=xqkT_sb[:, dc, sl],
                        start=dc == 0,
                        stop=dc == 5,
                    )
                nc.vector.tensor_scalar(
                    kt_sb[:, p, sl], psk, bk_sb[:, p : p + 1], None, ADD
                )

        _proj_qk(0)
        for n in range(NKC):
            psv = psum_z.tile([128, 512], F32, tag="z", name="psv")
            for dc in range(6):
                nc.tensor.matmul(
                    psv[:, :384],
                    lhsT=residT_sb[:, dc, 128 * n : 128 * n + 128],
                    rhs=wv_sb[:, dc, :],
                    start=dc == 0,
                    stop=dc == 5,
                )
            for h in range(HG):
                nc.vector.tensor_tensor(
                    v1_sb[:, n, 65 * h : 65 * h + 64],
                    psv[:, 64 * h : 64 * h + 64],
                    bv_sb[:, 64 * h : 64 * h + 64],
                    ADD,
                )
        if "c" not in phases:
            _proj_qk(1)
            _proj_qk(2)
            return

        # ---- phase C: attention, pair-outer; pair p+1's projections are
        # emitted after pair p's attention so the PE fills ACT-bound gaps
        # with projection work (their PSUM chains live in the z pool, so
        # they don't contend with the score tiles). Scores for the two
        # heads of a pair run CONCURRENTLY in the PE array: even head's
        # K=64 contraction uses array row-group 0 (partitions 0:64), odd
        # head's row-group 2 (partitions 64:128) — tile_position
        # auto-derives from the lhsT base partition. ----
        for p in range(NPAIR):
            if p > 0:
                _proj_qk(p)
            h0, h1 = 2 * p, 2 * p + 1
            # Flattened (j, m) slots with ONE-SLOT SOFTWARE PIPELINE: the
            # PE executes in strict program order, so the next slot's score
            # matmuls are emitted BEFORE this slot's z matmuls (which wait
            # on the exp) — otherwise every slot serializes on the ACT
            # latency. The per-j normalize is likewise deferred one slot so
            # its PE broadcast matmul never stalls on the fresh reciprocal.
            slots = [(j, m) for j in range(NJ) for m in range(j + 1)]

            def emit_st(j, m):
                qsl = slice(QC * j, QC * j + QC)
                kslA = slice(128 * 2 * m, 128 * 2 * m + 128)
                kslB = slice(128 * (2 * m + 1), 128 * (2 * m + 1) + 128)
                # one 2-bank tile holds both heads' score pair:
                # cols [h0@kcA | h0@kcB | h1@kcA | h1@kcB]
                sp = psum_st.tile([128, 4 * QC], F32, tag="st", name="sp")
                for ksl, c0 in ((kslA, 0), (kslB, QC)):
                    nc.tensor.matmul(
                        sp[:, c0 : c0 + QC],
                        lhsT=kt_sb[0:64, p, ksl],
                        rhs=qt_sb[0:64, p, qsl],
                        start=True,
                        stop=True,
                    )
                    nc.tensor.matmul(
                        sp[:, 2 * QC + c0 : 2 * QC + c0 + QC],
                        lhsT=kt_sb[64:128, p, ksl],
                        rhs=qt_sb[64:128, p, qsl],
                        start=True,
                        stop=True,
                    )
                return sp

            def emit_norm(j, zp0, zp1):
                qsl = slice(QC * j, QC * j + QC)
                # denominator is row 64 (cols 0:QC) of each z psum tile; a
                # K=1 ones outer-product broadcasts 1/denom into the tile's
                # spare columns, a DVE copy moves it to SBUF (single PSUM
                # read port), then one multiply per head
                for zp, slot in ((zp0, 0), (zp1, 1)):
                    rc_t = rc_pool.tile([65, QC], F32, tag="rc", name="rc_t")
                    rc = rc_t[64:65, :]
                    nc.vector.reciprocal(rc, zp[64:65, 0:QC])
                    nc.tensor.matmul(
                        zp[0:64, QC : 2 * QC],
                        lhsT=ones_sb[64:65, 0:64],
                        rhs=rc,
                        start=True,
                        stop=True,
                    )
                    bc = bc_pool.tile([64, QC], F32, tag="bc", name="bc")
                    nc.vector.tensor_copy(bc, zp[0:64, QC : 2 * QC])
                    if slot == 0:
                        nc.vector.tensor_tensor(
                            zt2_sb[0:64, p, qsl], zp[0:64, 0:QC], bc, MULT
                        )
                    else:
                        # odd head: stage at partitions 0:64, DMA-shift into
                        # rows 64:128 of the pair slot
                        zs = zs_pool.tile([64, QC], F16, tag="zs", name="zs")
                        nc.vector.tensor_tensor(zs, zp[0:64, 0:QC], bc, MULT)
                        nc.sync.dma_start(zt2_sb[64:128, p, qsl], zs)

            zps = {}
            pending = None
            sp_next = emit_st(*slots[0])
            for si, (j, m) in enumerate(slots):
                sp = sp_next
                if si + 1 < len(slots):
                    sp_next = emit_st(*slots[si + 1])
                if m == 0:
                    zp0 = psum_z.tile([65, 2 * QC], F32, tag="z", name="zp0")
                    zp1 = psum_z.tile([65, 2 * QC], F32, tag="z", name="zp1")
                    zps[j] = (zp0, zp1)
                zp0, zp1 = zps[j]
                pt = pt_pool.tile([128, 4 * QC], F16, tag="pt", name="pt")
                nc.scalar.activation(pt, sp, EXP)
                if m == j:  # diagonal kc pair: causal mask (both heads)
                    nc.vector.tensor_mul(pt, pt, mAB_sb)
                for kc, csl0, csl1 in (
                    (2 * m, slice(0, QC), slice(2 * QC, 3 * QC)),
                    (2 * m + 1, slice(QC, 2 * QC), slice(3 * QC, 4 * QC)),
                ):
                    nc.tensor.matmul(
                        zp0[:, 0:QC],
                        lhsT=v1_sb[:, kc, 65 * h0 : 65 * h0 + 65],
                        rhs=pt[:, csl0],
                        start=kc == 0,
                        stop=kc == 2 * j + 1,
                    )
                    nc.tensor.matmul(
                        zp1[:, 0:QC],
                        lhsT=v1_sb[:, kc, 65 * h1 : 65 * h1 + 65],
                        rhs=pt[:, csl1],
                        start=kc == 0,
                        stop=kc == 2 * j + 1,
                    )
                if pending is not None:
                    emit_norm(pending, *zps.pop(pending))
                    pending = None
                if m == j:
                    pending = j
            emit_norm(pending, *zps.pop(pending))

        # ---- phase D: output projection (after all pairs) ----
        if "d" not in phases:
            return
        for n in range(NKC):
            ps0 = psum_z.tile([128, 512], F32, tag="z", name="ps0")
            for t in range(3):
                nc.tensor.matmul(
                    ps0[:, :384],
                    lhsT=zt2_sb[:, t, 128 * n : 128 * n + 128],
                    rhs=wo_sb[:, t, 0:384],
                    start=t == 0,
                    stop=t == 2,
                )
            ob = out_pool.tile([128, D], F32, tag="out", name="ob")
            nc.vector.tensor_add(ob[:, 0:384], ps0[:, :384], bo_sb[:, 0:384])
            ps1 = psum_z.tile([128, 512], F32, tag="z", name="ps1")
            for t in range(3):
                nc.tensor.matmul(
                    ps1[:, :384],
                    lhsT=zt2_sb[:, t, 128 * n : 128 * n + 128],
                    rhs=wo_sb[:, t, 384:768],
                    start=t == 0,
                    stop=t == 2,
                )
            nc.vector.tensor_add(ob[:, 384:768], ps1[:, :384], bo_sb[:, 384:768])
            nc.sync.dma_start(out[128 * n : 128 * n + 128, :], ob)


def pack_inputs(
    resid_pre, shortformer_pos_embed, W_Q, W_K, W_V, W_O, b_Q, b_K, b_V, b_O
):
    """Build the 8 per-core input maps (host-side shard + transpose + cast)."""
    resid_pre = np.asarray(resid_pre, dtype=np.float32)
    pos = np.asarray(shortformer_pos_embed, dtype=np.float32)
    W_Q = np.asarray(W_Q, dtype=np.float32)
    W_K = np.asarray(W_K, dtype=np.float32)
    W_V = np.asarray(W_V, dtype=np.float32)
    W_O = np.asarray(W_O, dtype=np.float32)
    b_Q = np.asarray(b_Q, dtype=np.float32)
    b_K = np.asarray(b_K, dtype=np.float32)
    b_V = np.asarray(b_V, dtype=np.float32)
    b_O = np.asarray(b_O, dtype=np.float32)

    i_a = np.arange(128)[:, None]
    i_q = np.arange(QC)[None, :]
    maskA = (i_q >= i_a).astype(np.float16)
    maskB = (i_q >= i_a + 128).astype(np.float16)
    maskAB = np.concatenate([maskA, maskB, maskA, maskB], axis=1)

    # per-batch tensors (shared by the two head-group cores of each batch)
    residT_b = [
        np.ascontiguousarray(resid_pre[b].T).astype(np.float16) for b in range(B)
    ]
    xqkT_b = [
        np.ascontiguousarray((resid_pre[b] + pos[b]).T).astype(np.float16)
        for b in range(B)
    ]
    # per head-group tensors (shared by the four batch cores of each group)
    grp = []
    for g in range(2):
        hs = slice(HG * g, HG * g + HG)
        wq = np.concatenate([W_Q[h] for h in range(HG * g, HG * g + HG)], axis=1)
        wk = np.concatenate([W_K[h] / 8.0 for h in range(HG * g, HG * g + HG)], axis=1)
        wv = np.concatenate([W_V[h] for h in range(HG * g, HG * g + HG)], axis=1)
        wo = W_O[hs].reshape(HG * Dh, D)
        bqc = b_Q[hs].reshape(NPAIR, 128).T  # [128, 3]; col p = pair p biases
        bkc = (b_K[hs] / 8.0).reshape(NPAIR, 128).T
        bvc = np.broadcast_to(b_V[hs].reshape(1, HG * Dh), (128, HG * Dh))
        boc = np.broadcast_to(b_O[None, :] * 0.5, (128, D))
        grp.append(
            {
                "wq": wq.astype(np.float16),
                "wk": wk.astype(np.float16),
                "wv": wv.astype(np.float16),
                "wo": wo.astype(np.float16),
                "bq": np.ascontiguousarray(bqc),
                "bk": np.ascontiguousarray(bkc),
                "bv": np.ascontiguousarray(bvc),
                "bo": np.ascontiguousarray(boc),
                "maskAB": maskAB,
            }
        )
    in_maps = []
    for c in range(N_CORES):
        b, g = divmod(c, 2)
        in_maps.append({"residT": residT_b[b], "xqkT": xqkT_b[b], **grp[g]})
    return in_maps


class Runner:
    """Compiles the Bass module once (via the bass2jax PJRT path that
    bass_utils.run_bass_kernel_spmd uses under axon) and keeps the jitted
    sharded executable for reuse."""

    def __init__(self, reps=1, loop_reps=None, phases="bcd"):
        import jax
        from jax.sharding import Mesh, PartitionSpec
        from jax.experimental.shard_map import shard_map
        from concourse import bass2jax

        self.jax = jax
        bass2jax.install_neuronx_cc_hook()
        nc = build_nc(reps=reps, loop_reps=loop_reps, phases=phases)
        self.nc = nc

        partition_name = (
            nc.partition_id_tensor.name if nc.partition_id_tensor else None
        )
        in_names, out_names, out_avals, zero_outs = [], [], [], []
        for alloc in nc.m.functions[0].allocations:
            if not isinstance(alloc, mybir.MemoryLocationSet):
                continue
            name = alloc.memorylocations[0].name
            if alloc.kind == "ExternalInput":
                if name != partition_name:
                    in_names.append(name)
            elif alloc.kind == "ExternalOutput":
                shape = tuple(alloc.tensor_shape)
                dtype = mybir.dt.np(alloc.dtype)
                out_names.append(name)
                out_avals.append(jax.core.ShapedArray(shape, dtype))
                zero_outs.append(np.zeros(shape, dtype))
        self.in_names = list(in_names)
        self.out_names = out_names
        self.out_avals = out_avals
        self.zero_outs = zero_outs
        n_params = len(in_names)
        n_outs = len(out_names)
        all_in_names = in_names + out_names
        if partition_name is not None:
            all_in_names.append(partition_name)

        def _body(*args):
            operands = list(args)
            if partition_name is not None:
                operands.append(bass2jax.partition_id_tensor())
            outs = bass2jax._bass_exec_p.bind(
                *operands,
                out_avals=tuple(out_avals),
                in_names=tuple(all_in_names),
                out_names=tuple(out_names),
                lowering_input_output_aliases=(),
                sim_require_finite=True,
                sim_require_nnan=True,
                nc=nc,
            )
            return tuple(outs)

        devices = jax.devices()[:N_CORES]
        assert len(devices) == N_CORES, f"need {N_CORES} cores, have {len(devices)}"
        mesh = Mesh(np.asarray(devices), ("core",))
        in_specs = (PartitionSpec("core"),) * (n_params + n_outs)
        out_specs = (PartitionSpec("core"),) * n_outs
        donate = tuple(range(n_params, n_params + n_outs))
        self.sharded = jax.jit(
            shard_map(
                _body,
                mesh=mesh,
                in_specs=in_specs,
                out_specs=out_specs,
                check_rep=False,
            ),
            donate_argnums=donate,
            keep_unused=True,
        )

    def concat_inputs(self, in_maps):
        return [
            np.concatenate([np.asarray(m[name]) for m in in_maps], axis=0)
            for name in self.in_names
        ]

    def concat_zeros(self):
        return [
            np.zeros((N_CORES * z.shape[0], *z.shape[1:]), z.dtype)
            for z in self.zero_outs
        ]

    def run_concat(self, concat_in):
        """One sharded execution; returns the concatenated 'out' array
        [8*2048, 768] (per-core blocks along axis 0)."""
        out_arrs = self.sharded(*concat_in, *self.concat_zeros())
        return np.asarray(out_arrs[0])

    def run(self, in_maps):
        flat = self.run_concat(self.concat_inputs(in_maps))
        return flat.reshape(N_CORES, S, D)


_RUNNER = None


def get_runner():
    global _RUNNER
    if _RUNNER is None:
        _RUNNER = Runner()
    return _RUNNER


def kernel(**inputs):
    runner = get_runner()
    in_maps = pack_inputs(**inputs)
    per_core = runner.run(in_maps)
    # host all-reduce of the two head-group partials per batch
    out = per_core[0::2] + per_core[1::2]  # [4, 2048, 768]
    return out.astype(np.float32)


if __name__ == "__main__":
    # quick self-run with random inputs
    rng = np.random.default_rng(0)
    ins = {
        "resid_pre": rng.standard_normal((B, S, D), dtype=np.float32),
        "shortformer_pos_embed": rng.standard_normal((B, S, D), dtype=np.float32),
        "W_Q": rng.standard_normal((H, D, Dh), dtype=np.float32) * 0.02,
        "W_K": rng.standard_normal((H, D, Dh), dtype=np.float32) * 0.02,
        "W_V": rng.standard_normal((H, D, Dh), dtype=np.float32) * 0.02,
        "W_O": rng.standard_normal((H, Dh, D), dtype=np.float32) * 0.02,
        "b_Q": np.zeros((H, Dh), np.float32),
        "b_K": np.zeros((H, Dh), np.float32),
        "b_V": np.zeros((H, Dh), np.float32),
        "b_O": np.zeros((D,), np.float32),
    }
    out = kernel(**ins)
    print("out", out.shape, out.dtype, float(np.abs(out).max()))
